# revision 9
# baseline (speedup 1.0000x reference)
"""BertSelfAttention Bass/Tile kernel for 8 Trainium2 NeuronCores.

Sharding: data-parallel over batch (B=8 -> 1 batch per core); weights and
attention mask are replicated to every core.

Per-core dataflow (batch b), fp32 inputs, float32r (1-cycle/row fp32 mode)
matmuls:
  hidden^T, Wq^T, Wk^T, Wv^T via PE transposes (exact fp32), rounded to
  f32r in the PSUM->SBUF copy.
  Q^T = (Wq^T).T @ h^T  [d_out, s]   (bias added per-partition on DVE)
  K^T likewise; V computed in [s, d_out] layout (bias via K=1 ones matmul).
  Per (head, q-chunk of 512):
    scores^T[k, q] = K_h Q_h^T / 8 on PE (contraction d=64)
    P = exp(scores/8 + maskbias[k]) on ScalarE (mask fused as bias; -30000
        for masked keys -> exp == 0 exactly)
    ctx^T[d, q] (+ ones row -> softmax sums) = [V_h | 1]^T @ P, PE accum
    inv = 1/sums; PE-broadcast inv across partitions; DVE multiplies give
    normalized probs (streamed to DRAM as [h, k, q]) and context^T [h, d, q].
Host side: transpose-gather the per-core outputs into the reference layouts.
"""

import os
import sys
import types

import numpy as np

# ---------------------------------------------------------------------------
# axon NTFF-profile hook shim: the container's `antenv` package lacks
# `axon_hooks`, so install an equivalent module before importing concourse.
# This only enables profiling (trace=True); runs work without it.
# ---------------------------------------------------------------------------
if "antenv.axon_hooks" not in sys.modules:
    _mod = types.ModuleType("antenv.axon_hooks")
    _mod._hook = None

    def _set_hook(h, _m=_mod):
        _m._hook = h

    def _get_hook(_m=_mod):
        return _m._hook

    _mod.set_axon_ntff_profile_hook = _set_hook
    _mod.get_axon_ntff_profile_hook = _get_hook
    sys.modules["antenv.axon_hooks"] = _mod
    try:
        from trn_agent_boot.trn_boot import _ntff_profile_via_ctypes

        _set_hook(_ntff_profile_via_ctypes("/opt/axon/libaxon_pjrt.so"))
    except Exception:
        pass

import concourse.bass as bass  # noqa: E402
import concourse.mybir as mybir  # noqa: E402
import concourse.tile as tile  # noqa: E402
from concourse import bacc  # noqa: E402
from concourse.bass_utils import run_bass_kernel_spmd  # noqa: E402
from concourse.masks import make_identity  # noqa: E402

F32 = mybir.dt.float32
F32R = mybir.dt.float32r
AF = mybir.ActivationFunctionType

B, S, D = 8, 1024, 1024
H = 16
HD = D // H  # 64
N_CORES = 8
P = 128
QC = 512  # q-chunk (one PSUM bank of fp32)
NEG = -30000.0  # additive mask bias; exp(x + NEG) == 0 exactly in fp32
KT = D // P  # 8 k-tiles of 128
ST = S // P  # 8 s-tiles of 128


def build_kernel():
    nc = bacc.Bacc("TRN2", target_bir_lowering=False, debug=False)

    hidden = nc.dram_tensor("hidden", [S, D], F32, kind="ExternalInput").ap()
    wq = nc.dram_tensor("wq", [D, D], F32, kind="ExternalInput").ap()
    wk = nc.dram_tensor("wk", [D, D], F32, kind="ExternalInput").ap()
    wv = nc.dram_tensor("wv", [D, D], F32, kind="ExternalInput").ap()
    bq = nc.dram_tensor("bq", [D], F32, kind="ExternalInput").ap()
    bk = nc.dram_tensor("bk", [D], F32, kind="ExternalInput").ap()
    bv = nc.dram_tensor("bv", [D], F32, kind="ExternalInput").ap()
    maskb = nc.dram_tensor("maskb", [S], F32, kind="ExternalInput").ap()

    probs_t = nc.dram_tensor("probs_t", [H, S, S], F32, kind="ExternalOutput").ap()
    ctx_t = nc.dram_tensor("ctx_t", [H, HD, S], F32, kind="ExternalOutput").ap()

    with tile.TileContext(nc) as tc:
        const_pool = tc.alloc_tile_pool(name="consts", bufs=1)
        big_pool = tc.alloc_tile_pool(name="big", bufs=1)

        ident = const_pool.tile([P, P], F32, tag="ident")
        make_identity(nc, ident[:])
        ones_f32 = const_pool.tile([P, 16], F32, tag="ones_f32")
        nc.vector.memset(ones_f32[:], 1.0)
        ones_f32row = const_pool.tile([1, P], F32, tag="ones_f32row")
        nc.vector.memset(ones_f32row[:], 1.0)
        ones_row = const_pool.tile([1, P], F32R, tag="ones_row")
        nc.vector.tensor_copy(ones_row[:], ones_f32row[:])

        # per-partition bias/mask tiles
        bq_sb = [const_pool.tile([P, 1], F32, name=f"bq{m}", tag=f"bq{m}") for m in range(KT)]
        bk_sb = [const_pool.tile([P, 1], F32, name=f"bk{m}", tag=f"bk{m}") for m in range(KT)]
        mb_sb = [const_pool.tile([P, 1], F32, name=f"mb{k}", tag=f"mb{k}") for k in range(KT)]
        for m in range(KT):
            nc.sync.dma_start(bq_sb[m][:], bq[bass.ts(m, P)].rearrange("(p o) -> p o", o=1))
            nc.sync.dma_start(bk_sb[m][:], bk[bass.ts(m, P)].rearrange("(p o) -> p o", o=1))
            nc.sync.dma_start(mb_sb[m][:], maskb[bass.ts(m, P)].rearrange("(p o) -> p o", o=1))
        bv_st = const_pool.tile([1, D], F32, tag="bv_st")
        nc.sync.dma_start(bv_st[:], bv.rearrange("(o d) -> o d", o=1))
        bv_sb = const_pool.tile([1, D], F32R, tag="bv_sb")
        nc.vector.tensor_copy(bv_sb[:], bv_st[:])

        # persistent activations
        hT = [big_pool.tile([P, S], F32R, name=f"hT{k}", tag=f"hT{k}") for k in range(KT)]
        QT = [big_pool.tile([P, S], F32R, name=f"QT{m}", tag=f"QT{m}") for m in range(KT)]
        KTt = [big_pool.tile([P, S], F32R, name=f"KT{m}", tag=f"KT{m}") for m in range(KT)]
        # V in [s, d] layout, 65 cols per head (64 data + ones col)
        V_sb = [big_pool.tile([P, H * (HD + 1)], F32R, name=f"V{s}", tag=f"V{s}") for s in range(ST)]
        for s in range(ST):
            col = V_sb[s][:].rearrange("p (h e) -> p h e", e=HD + 1)
            nc.vector.tensor_copy(
                col[:, :, HD : HD + 1],
                ones_f32[:].rearrange("p (h e) -> p h e", e=1),
            )

        # ---------------- phase A/B: transposes + projections ----------------
        with (
            tc.tile_pool(name="stage", bufs=3) as stage_pool,
            tc.tile_pool(name="wt", bufs=1) as wt_pool,
            tc.tile_pool(name="tp_ps", bufs=4, space="PSUM") as tp_ps,
            tc.tile_pool(name="proj_ps", bufs=3, space="PSUM") as proj_ps,
        ):
            def transpose_into(dst_tiles, src_dram):
                # src [row, col] DRAM -> dst_tiles[ct][:, row-block] = src.T
                for rt in range(ST):
                    nat = stage_pool.tile([P, D], F32, tag="nat")
                    nc.sync.dma_start(nat[:], src_dram[bass.ts(rt, P), :])
                    for ct in range(KT):
                        ps = tp_ps.tile([P, P], F32, tag="tp")
                        nc.tensor.transpose(ps[:], nat[:, bass.ts(ct, P)], ident[:])
                        nc.vector.tensor_copy(
                            dst_tiles[ct][:, bass.ts(rt, P)], ps[:]
                        )

            transpose_into(hT, hidden)

            WT = [wt_pool.tile([P, D], F32R, name=f"WT{k}", tag=f"WT{k}") for k in range(KT)]

            # --- Q^T and K^T: out[m-tile, s-chunk] = sum_k WT[k][:,m] . hT[k][:,s]
            for w_dram, out_tiles, bias_sb in ((wq, QT, bq_sb), (wk, KTt, bk_sb)):
                transpose_into(WT, w_dram)
                for mt in range(KT):
                    for sc in range(S // QC):
                        ps = proj_ps.tile([P, QC], F32, tag="proj")
                        for k in range(KT):
                            nc.tensor.matmul(
                                ps[:],
                                WT[k][:, bass.ts(mt, P)],
                                hT[k][:, bass.ts(sc, QC)],
                                start=(k == 0),
                                stop=(k == KT - 1),
                            )
                        nc.vector.tensor_scalar_add(
                            out_tiles[mt][:, bass.ts(sc, QC)], ps[:], bias_sb[mt]
                        )

            # --- V: out[s-tile, m-chunk] = sum_k hT[k][:,s] . WT[k][:,m] + bv
            transpose_into(WT, wv)
            for st in range(ST):
                for mc in range(D // QC):
                    ps = proj_ps.tile([P, QC], F32, tag="proj")
                    for k in range(KT):
                        nc.tensor.matmul(
                            ps[:],
                            hT[k][:, bass.ts(st, P)],
                            WT[k][:, bass.ts(mc, QC)],
                            start=(k == 0),
                            stop=False,
                        )
                    nc.tensor.matmul(
                        ps[:],
                        ones_row[:],
                        bv_sb[:, bass.ts(mc, QC)],
                        start=False,
                        stop=True,
                    )
                    # scatter the 8 heads of this 512-chunk into the 65-stride layout
                    dst = V_sb[st][:].rearrange("p (h e) -> p h e", e=HD + 1)
                    heads = QC // HD
                    nc.vector.tensor_copy(
                        dst[:, mc * heads : (mc + 1) * heads, 0:HD],
                        ps[:].rearrange("p (h e) -> p h e", e=HD),
                    )

        # ---------------- phase C: attention ----------------
        with (
            tc.tile_pool(name="p_sb", bufs=2) as p_pool,
            tc.tile_pool(name="pn_sb", bufs=6) as pn_pool,
            tc.tile_pool(name="misc_sb", bufs=3) as misc_pool,
            tc.tile_pool(name="sc_ps", bufs=3, space="PSUM") as sc_ps_pool,
            tc.tile_pool(name="ctx_ps", bufs=2, space="PSUM") as ctx_ps_pool,
            tc.tile_pool(name="bc_ps", bufs=2, space="PSUM") as bc_ps_pool,
        ):
            for h in range(H):
                mt_h, off = divmod(h, 2)
                r0 = off * HD
                for qc in range(S // QC):
                    qs = bass.ts(qc, QC)
                    ctx = ctx_ps_pool.tile([HD + 1, QC], F32, tag="ctx")
                    p_tiles = []
                    for k in range(KT):
                        sc = sc_ps_pool.tile([P, QC], F32, tag="sc")
                        nc.tensor.matmul(
                            sc[:],
                            KTt[mt_h][r0 : r0 + HD, bass.ts(k, P)],
                            QT[mt_h][r0 : r0 + HD, qs],
                            start=True,
                            stop=True,
                        )
                        pt = p_pool.tile([P, QC], F32R, tag=f"P{k}")
                        nc.scalar.activation(
                            pt[:], sc[:], AF.Exp, bias=mb_sb[k][:], scale=0.125
                        )
                        nc.tensor.matmul(
                            ctx[:],
                            V_sb[k][:, h * (HD + 1) : (h + 1) * (HD + 1)],
                            pt[:],
                            start=(k == 0),
                            stop=(k == KT - 1),
                        )
                        p_tiles.append(pt)

                    inv = misc_pool.tile([1, QC], F32R, tag="inv")
                    with nc.allow_low_precision(
                        reason="f32r round of softmax reciprocal"
                    ):
                        nc.vector.reciprocal(inv[:], ctx[HD : HD + 1, :])
                    bc = bc_ps_pool.tile([P, QC], F32, tag="bc")
                    nc.tensor.matmul(bc[:], ones_row[:], inv[:], start=True, stop=True)

                    for k in range(KT):
                        pn = pn_pool.tile([P, QC], F32, tag="pn")
                        nc.vector.tensor_mul(
                            pn[:], p_tiles[k].bitcast(F32)[:], bc[:]
                        )
                        nc.sync.dma_start(probs_t[h, bass.ts(k, P), qs], pn[:])

                    cu = misc_pool.tile([HD, QC], F32, tag="cu")
                    nc.scalar.copy(cu[:], ctx[0:HD, :])
                    cn = misc_pool.tile([HD, QC], F32, tag="cn")
                    nc.vector.tensor_mul(cn[:], cu[:], bc[0:HD, :])
                    nc.sync.dma_start(ctx_t[h, :, qs], cn[:])

        big_pool.release()
        const_pool.release()

    nc.compile()
    return nc


_NC_CACHE = None


def _get_nc():
    global _NC_CACHE
    if _NC_CACHE is None:
        _NC_CACHE = build_kernel()
    return _NC_CACHE


def kernel(hidden_states, attention_mask, Wq, bq, Wk, bk, Wv, bv, trace=False):
    hidden_states = np.ascontiguousarray(hidden_states, dtype=np.float32)
    Wq = np.ascontiguousarray(Wq, dtype=np.float32)
    Wk = np.ascontiguousarray(Wk, dtype=np.float32)
    Wv = np.ascontiguousarray(Wv, dtype=np.float32)
    bq = np.ascontiguousarray(bq, dtype=np.float32)
    bk = np.ascontiguousarray(bk, dtype=np.float32)
    bv = np.ascontiguousarray(bv, dtype=np.float32)
    mask = np.asarray(attention_mask).reshape(B, S)
    maskb = np.where(mask != 0, np.float32(0.0), np.float32(NEG))

    nc = _get_nc()
    in_maps = [
        {
            "hidden": hidden_states[b],
            "wq": Wq,
            "wk": Wk,
            "wv": Wv,
            "bq": bq,
            "bk": bk,
            "bv": bv,
            "maskb": maskb[b],
        }
        for b in range(B)
    ]
    res = run_bass_kernel_spmd(
        nc, in_maps, core_ids=list(range(N_CORES)), trace=trace
    )
    probs = np.empty((B, H, S, S), dtype=np.float32)
    context = np.empty((B, S, D), dtype=np.float32)
    for b in range(B):
        r = res.results[b]
        probs[b] = r["probs_t"].transpose(0, 2, 1)  # [h,k,q] -> [h,q,k]
        context[b] = (
            r["ctx_t"].transpose(2, 0, 1).reshape(S, D)  # [h,d,q] -> [q, h*d]
        )
    if trace:
        kernel.last_exec_time_ns = res.exec_time_ns
        kernel.last_results = res
    return context, probs


if __name__ == "__main__":
    rng = np.random.default_rng(0)
    inputs = {
        "hidden_states": rng.standard_normal((B, S, D), dtype=np.float32),
        "attention_mask": (rng.integers(0, 2, (B, 1, 1, S))).astype(np.int32),
        "Wq": (rng.standard_normal((D, D), dtype=np.float32) / 32.0),
        "bq": np.zeros(D, np.float32),
        "Wk": (rng.standard_normal((D, D), dtype=np.float32) / 32.0),
        "bk": np.zeros(D, np.float32),
        "Wv": (rng.standard_normal((D, D), dtype=np.float32) / 32.0),
        "bv": np.zeros(D, np.float32),
    }
    ctx, probs = kernel(**inputs)
    print("ran:", ctx.shape, probs.shape)
